# revision 13
# baseline (speedup 1.0000x reference)
"""Trainium2 Bass kernel for GQA attention with int8-quantized QK^T (8-core SPMD).

Reference (per-core shard c of 8):
  q = x @ Wq.T          -> heads [4c..4c+4), computed as q[t, 256]
  k = x @ Wk.T, v = x @ Wv.T  -> kv-head c, [t, 64] each
  per-token-per-head int8 absmax quantization of q, k (exact emulation:
  integer values live in bf16 -- integers <= 127 are exact in bf16, and the
  i8xi8 -> i32 dot over 64 terms (<2^24) is exact in f32 PSUM accumulate)
  scoresT[t2, t1] = k_i8.T @ q_i8 ; dequant = scoresT * ksr[t2] * qsr[t1]
  p = exp(dequant) (no max-subtraction; |arg| <= ~58 is safe in f32)
  attT[hd, t1] = v_aug.T @ p  with ones column -> row 64 = sumexp
  normalize, AllGather heads across cores, out_c = WoT_c.T @ attT_full
  (o_proj is column-sharded -> host concatenates; no AllReduce needed)

Layouts are transposed throughout ([feature, token]) so softmax runs along
the free axis of nothing -- the only transposes are the 128-wide PE
transposes of q_i8/k_i8/qsr after quantization (natural layout is needed
for the per-token absmax along the free axis).
"""

import numpy as np
import ml_dtypes
from contextlib import ExitStack

import concourse.bass as bass
import concourse.mybir as mybir
import concourse.tile as tile
from concourse import bacc
from concourse.bass import ts, ds
from concourse.masks import make_identity

NCORES = 8
P = 128
S = 2048          # tokens
D = 2048          # model dim
HD = 64           # head dim
NHL = 4           # q heads per core
JQ = NHL * HD     # 256 (q cols per core)
NQK = JQ + HD     # 320 (q + k cols, the quantized part)
NQKV = JQ + 2 * HD  # 384
TT = S // P       # 16 token tiles
DT = D // P       # 16 d tiles
NB = 4            # t1 blocks
BN = S // NB      # 512
MAGIC = 12582912.0  # 1.5 * 2**23: (x + MAGIC) - MAGIC == round-half-even(x)
SM = HD ** -0.5   # 0.125
F32 = mybir.dt.float32
F32R = mybir.dt.float32r
BF16 = mybir.dt.bfloat16
AF = mybir.ActivationFunctionType
ALU = mybir.AluOpType


def build_nc(debug_taps=False):
    nc = bacc.Bacc(target_bir_lowering=False, debug=False, num_devices=NCORES)
    xT = nc.declare_dram_parameter("xT", [D, S], F32R, isOutput=False)
    wqkv = nc.declare_dram_parameter("wqkv", [D, NQKV], F32R, isOutput=False)
    woT = nc.declare_dram_parameter("woT", [D, JQ], BF16, isOutput=False)
    tri = nc.declare_dram_parameter("tri", [P, P], BF16, isOutput=False)
    out_ext = nc.declare_dram_parameter("out", [JQ, S], F32, isOutput=True)

    taps = None
    if debug_taps:
        taps = {
            "qT_d": nc.declare_dram_parameter("qT_d", [P, 2, S], BF16, isOutput=True),
            "kT_d": nc.declare_dram_parameter("kT_d", [P, S], BF16, isOutput=True),
            "v_d": nc.declare_dram_parameter("v_d", [P, TT, HD + 1], BF16, isOutput=True),
            "ksr_d": nc.declare_dram_parameter("ksr_d", [P, TT], F32, isOutput=True),
            "qsrT_d": nc.declare_dram_parameter("qsrT_d", [97, S], F32, isOutput=True),
            "att_d": nc.declare_dram_parameter("att_d", [JQ, BN], BF16, isOutput=True),
            "attf_d": nc.declare_dram_parameter("attf_d", [NCORES * JQ, BN], BF16, isOutput=True),
        }
    with tile.TileContext(nc) as tc:
        with ExitStack() as ctx:
            _body(nc, tc, ctx, xT, wqkv, woT, tri, out_ext, taps)
    nc.finalize()
    return nc


def _body(nc, tc, ctx, xT, wqkv, woT, tri, out_ext, taps=None):
    # DRAM bounce buffers for the AllGather (one per t1 block)
    dram_pool = ctx.enter_context(tc.tile_pool(name="dram", bufs=1, space="DRAM"))
    att_shard = [
        dram_pool.tile([JQ, BN], BF16, name=f"att_shard{b}", tag=f"as{b}")
        for b in range(NB)
    ]
    att_full = [
        dram_pool.tile([NCORES * JQ, BN], BF16, addr_space="Shared",
                       name=f"att_full{b}", tag=f"af{b}")
        for b in range(NB)
    ]

    singles = ctx.enter_context(tc.tile_pool(name="singles", bufs=1))
    xpool = ctx.enter_context(tc.tile_pool(name="xpool", bufs=3))
    quant = ctx.enter_context(tc.tile_pool(name="quant", bufs=3))
    ei_pool = ctx.enter_context(tc.tile_pool(name="ei", bufs=4))
    p_pool = ctx.enter_context(tc.tile_pool(name="pp", bufs=4))
    bc_sb = ctx.enter_context(tc.tile_pool(name="bc_sb", bufs=3))
    an_sb = ctx.enter_context(tc.tile_pool(name="an_sb", bufs=3))
    orhs = ctx.enter_context(tc.tile_pool(name="orhs", bufs=4))
    osb = ctx.enter_context(tc.tile_pool(name="osb", bufs=3))
    # PSUM pools (8 banks of 2KB/partition total)
    ps_main = ctx.enter_context(tc.tile_pool(name="ps_main", bufs=2, space="PSUM"))
    ps_acc = ctx.enter_context(tc.tile_pool(name="ps_acc", bufs=4, space="PSUM"))
    ps_aux = ctx.enter_context(tc.tile_pool(name="ps_aux", bufs=2, space="PSUM"))

    # ---------------- persistent tiles ----------------
    wqkv_sb = singles.tile([P, DT, NQKV], F32R)
    nc.sync.dma_start(out=wqkv_sb, in_=wqkv.rearrange("(a p) n -> p a n", p=P))
    woT_sb = singles.tile([P, DT, JQ], BF16)
    nc.sync.dma_start(out=woT_sb, in_=woT.rearrange("(a p) n -> p a n", p=P))
    tri_sb = singles.tile([P, P], BF16)
    nc.sync.dma_start(out=tri_sb, in_=tri[:, :])
    id_bf = singles.tile([P, P], BF16)
    make_identity(nc, id_bf)
    id_f32 = singles.tile([P, P], F32)
    make_identity(nc, id_f32)
    qT_sb = singles.tile([P, 2, S], BF16)   # [64*hh+hd, pair, t]
    kT_sb = singles.tile([P, S], BF16)      # k dup'd on both partition halves
    v_sb = singles.tile([P, TT, HD + 1], BF16)
    nc.vector.memset(v_sb, 1.0)             # col 64 stays 1.0 (sumexp trick)
    ksr_sb = singles.tile([P, TT], F32)     # amax_k * SM/127 per t2 tile
    qsrT_sb = singles.tile([97, S], F32)    # amax_q/127, head h at partition 32h
    qsrT4 = singles.tile([1, NHL, S], F32)  # same rows, moved to partition 0

    # ---------------- phase B: qkv projection + quantization ----------------
    for i in range(TT):
        xcol = xpool.tile([P, DT, P], F32R, tag="xcol")
        nc.sync.dma_start(out=xcol, in_=xT[:, ts(i, P)].rearrange("(a p) m -> p a m", p=P))
        qkv = ps_main.tile([P, NQKV], F32, tag="mm")
        for d in range(DT):
            nc.tensor.matmul(
                qkv, lhsT=xcol[:, d, :], rhs=wqkv_sb[:, d, :],
                start=(d == 0), stop=(d == DT - 1))
        # v -> bf16 (ones column at 64 preset)
        nc.any.tensor_copy(v_sb[:, i, 0:HD], qkv[:, NQK:NQKV])
        # absmax over each head group of 64 (q heads 0-3, k group 4)
        amax = quant.tile([P, 5], F32, tag="amax")
        nc.vector.tensor_reduce(
            amax, qkv[:, 0:NQK].rearrange("p (g h) -> p g h", h=HD),
            axis=mybir.AxisListType.X, op=ALU.max, apply_absolute_value=True)
        amax_c = quant.tile([P, 5], F32, tag="amaxc")
        nc.vector.tensor_scalar_max(amax_c, amax, 1e-6)
        rec = quant.tile([P, 5], F32, tag="rec")
        nc.vector.reciprocal(rec, amax_c)
        scl = quant.tile([P, 5], F32, tag="scl")
        nc.vector.tensor_scalar_mul(scl, rec, 127.0)
        qsr = quant.tile([P, 97], F32, tag="qsr")
        nc.vector.memset(qsr, 0.0)
        qsr_strided = bass.AP(tensor=qsr.tensor, offset=qsr.offset,
                              ap=[qsr.ap[0], [32, NHL]])
        nc.vector.tensor_scalar_mul(qsr_strided, amax_c[:, 0:NHL], 1.0 / 127.0)
        nc.vector.tensor_scalar_mul(ksr_sb[:, i:i + 1], amax_c[:, 4:5], SM / 127.0)
        # round-to-int via magic number: ACT does q*scl + MAGIC, DVE does -MAGIC
        tmp = quant.tile([P, NQK], F32, tag="tmp")
        for h in range(5):
            nc.scalar.activation(
                out=tmp[:, ts(h, HD)], in_=qkv[:, ts(h, HD)],
                func=AF.Copy, scale=scl[:, h:h + 1], bias=MAGIC)
        qki = quant.tile([P, NQK], BF16, tag="qki")
        nc.vector.tensor_scalar_add(qki, tmp, -MAGIC)
        # transposes: q halves -> qT pairs; k -> kT; qsr -> qsrT rows
        for half in range(2):
            tp = ps_aux.tile([P, P], BF16, tag="aux")
            nc.tensor.transpose(tp, qki[:, ts(half, P)], id_bf)
            nc.any.tensor_copy(qT_sb[:, half, ts(i, P)], tp)
        tpk = ps_aux.tile([HD, P], BF16, tag="aux")
        nc.tensor.transpose(tpk, qki[:, JQ:NQK], id_bf)
        nc.any.tensor_copy(kT_sb[0:HD, ts(i, P)], tpk)
        tpq = ps_aux.tile([97, P], F32, tag="aux")
        nc.tensor.transpose(tpq, qsr, id_f32)
        nc.any.tensor_copy(qsrT_sb[:, ts(i, P)], tpq)
    # duplicate k rows into partitions 64..127 (for row-group packed matmuls)
    nc.sync.dma_start(out=kT_sb[HD:P, :], in_=kT_sb[0:HD, :])
    # partition_broadcast only reads partition 0 correctly on HW: move the
    # qsr rows (at partitions 32h) down to partition 0 via SBUF->SBUF DMA
    for h in range(NHL):
        nc.sync.dma_start(out=qsrT4[0:1, h, :], in_=qsrT_sb[32 * h:32 * h + 1, :])

    if taps is not None:
        nc.sync.dma_start(out=taps["qT_d"][:, :, :], in_=qT_sb)
        nc.sync.dma_start(out=taps["kT_d"][:, :], in_=kT_sb)
        nc.sync.dma_start(out=taps["v_d"][:, :, :], in_=v_sb)
        nc.sync.dma_start(out=taps["ksr_d"][:, :], in_=ksr_sb)
        nc.sync.dma_start(out=taps["qsrT_d"][:, :], in_=qsrT_sb)

    # ---------------- phase C/D: attention blocks + AG + o_proj ----------------
    for b in range(NB):
        na = 4 * (b + 1)
        for pair in range(2):
            heads = (2 * pair, 2 * pair + 1)
            # broadcast qsr rows across 128 partitions via K=1 matmul
            qbc = []
            for hh, h in enumerate(heads):
                bcs = bc_sb.tile([P, BN], F32, tag="bcs")
                nc.gpsimd.partition_broadcast(
                    bcs, qsrT4[0:1, h, ts(b, BN)], channels=P)
                qbc.append(bcs)
            atps = [ps_acc.tile([HD + 1, BN], F32, tag="acc", name=f"at{b}_{pair}_{hh}") for hh in range(2)]
            for a in range(na):
                arel = a - 4 * b
                off = max(0, arel) * P
                n_sub = BN - off
                for hh, h in enumerate(heads):
                    rows = slice(HD * hh, HD * hh + HD)
                    sc = ps_main.tile([P, BN], F32, tag="mm")
                    nc.tensor.matmul(
                        sc[:, off:], lhsT=kT_sb[rows, ts(a, P)],
                        rhs=qT_sb[rows, pair, ds(b * BN + off, n_sub)],
                        start=True, stop=True)
                    ei = ei_pool.tile([P, BN], F32, tag="ei")
                    nc.vector.tensor_tensor(
                        ei[:, off:], sc[:, off:], qbc[hh][:, off:], op=ALU.mult)
                    pt = p_pool.tile([P, BN], BF16, tag="pt")
                    nc.scalar.activation(
                        out=pt[:, off:], in_=ei[:, off:], func=AF.Exp,
                        scale=ksr_sb[:, a:a + 1])
                    if arel >= 0:
                        nc.vector.tensor_mul(
                            pt[:, off:off + P], pt[:, off:off + P], tri_sb)
                    nc.tensor.matmul(
                        atps[hh][:, off:], lhsT=v_sb[:, a, :], rhs=pt[:, off:],
                        start=(a == 0), stop=(a == na - 1))
            # normalize by sumexp (row 64) and emit bf16 shard
            for hh, h in enumerate(heads):
                rcp = bc_sb.tile([65, BN], F32, tag="rcp")
                nc.vector.reciprocal(rcp[HD:HD + 1, :], atps[hh][HD:HD + 1, :])
                rcp0 = bc_sb.tile([1, BN], F32, tag="rcp0")
                nc.sync.dma_start(out=rcp0[0:1, :], in_=rcp[HD:HD + 1, :])
                rbs = bc_sb.tile([HD, BN], F32, tag="rbs")
                nc.gpsimd.partition_broadcast(
                    rbs, rcp0[0:1, :], channels=HD)
                ans = an_sb.tile([HD, BN], BF16, tag="ans")
                nc.vector.tensor_mul(ans, atps[hh][0:HD, :], rbs)
                nc.sync.dma_start(out=att_shard[b][ts(h, HD), :], in_=ans)
        nc.gpsimd.collective_compute(
            "AllGather", ALU.bypass,
            replica_groups=[list(range(NCORES))],
            ins=[att_shard[b][:, :]], outs=[att_full[b][:, :]])
        if taps is not None and b == 0:
            nc.sync.dma_start(out=taps["att_d"][:, :], in_=att_shard[0][:, :])
            nc.sync.dma_start(out=taps["attf_d"][:, :], in_=att_full[0][:, :])
        # o_proj for this t1 block (column shard of Wo -> no reduce needed)
        oph = [ps_acc.tile([P, BN], F32, tag="acc", name=f"op{b}_{m}") for m in range(2)]
        for j in range(DT):
            rt = orhs.tile([P, BN], BF16, tag="rt")
            nc.sync.dma_start(out=rt, in_=att_full[b][ts(j, P), :])
            for m in range(2):
                nc.tensor.matmul(
                    oph[m], lhsT=woT_sb[:, j, ts(m, P)], rhs=rt,
                    start=(j == 0), stop=(j == DT - 1))
        for m in range(2):
            ot = osb.tile([P, BN], F32, tag="ot")
            nc.any.tensor_copy(ot, oph[m])
            nc.sync.dma_start(out=out_ext[ts(m, P), ts(b, BN)], in_=ot)


# ---------------- host side ----------------

def prep_in_maps(x, Wq, Wk, Wv, Wo):
    bf = ml_dtypes.bfloat16
    xTh = np.ascontiguousarray(x.reshape(S, D).T.astype(np.float32))
    tri_h = np.ascontiguousarray(
        (np.arange(P)[:, None] <= np.arange(P)[None, :]).astype(bf))
    in_maps = []
    for c in range(NCORES):
        wq = Wq[c * JQ:(c + 1) * JQ, :].T
        wk = Wk[c * HD:(c + 1) * HD, :].T
        wv = Wv[c * HD:(c + 1) * HD, :].T
        wqkv_h = np.ascontiguousarray(
            np.concatenate([wq, wk, wv], axis=1).astype(np.float32))
        woT_h = np.ascontiguousarray(
            Wo[c * JQ:(c + 1) * JQ, :].T.astype(bf))
        in_maps.append({"xT": xTh, "wqkv": wqkv_h, "woT": woT_h, "tri": tri_h})
    return in_maps


def unshard(results):
    out = np.empty((S, D), dtype=np.float32)
    for c in range(NCORES):
        out[:, c * JQ:(c + 1) * JQ] = results[c]["out"].T
    return out.reshape(1, S, D)


def kernel(x, Wq, Wk, Wv, Wo):
    from concourse.bass_utils import run_bass_kernel_spmd
    nc = build_nc()
    in_maps = prep_in_maps(x, Wq, Wk, Wv, Wo)
    res = run_bass_kernel_spmd(nc, in_maps, core_ids=list(range(NCORES)))
    return unshard(res.results)


# revision 15
# speedup vs baseline: 1.0949x; 1.0949x over previous
"""Trainium2 Bass kernel for GQA attention with int8-quantized QK^T (8-core SPMD).

Reference (per-core shard c of 8):
  q = x @ Wq.T          -> heads [4c..4c+4), computed as q[t, 256]
  k = x @ Wk.T, v = x @ Wv.T  -> kv-head c, [t, 64] each
  per-token-per-head int8 absmax quantization of q, k (exact emulation:
  integer values live in bf16 -- integers <= 127 are exact in bf16, and the
  i8xi8 -> i32 dot over 64 terms (<2^24) is exact in f32 PSUM accumulate)
  scoresT[t2, t1] = k_i8.T @ q_i8 ; dequant = scoresT * ksr[t2] * qsr[t1]
  p = exp(dequant) (no max-subtraction; |arg| <= ~58 is safe in f32)
  attT[hd, t1] = v_aug.T @ p  with ones column -> row 64 = sumexp
  normalize, AllGather heads across cores, out_c = WoT_c.T @ attT_full
  (o_proj is column-sharded -> host concatenates; no AllReduce needed)

Layouts are transposed throughout ([feature, token]) so softmax runs along
the free axis of nothing -- the only transposes are the 128-wide PE
transposes of q_i8/k_i8/qsr after quantization (natural layout is needed
for the per-token absmax along the free axis).
"""

import numpy as np
import ml_dtypes
from contextlib import ExitStack

import concourse.bass as bass
import concourse.mybir as mybir
import concourse.tile as tile
from concourse import bacc
from concourse.bass import ts, ds
from concourse.masks import make_identity

NCORES = 8
P = 128
S = 2048          # tokens
D = 2048          # model dim
HD = 64           # head dim
NHL = 4           # q heads per core
JQ = NHL * HD     # 256 (q cols per core)
NQK = JQ + HD     # 320 (q + k cols, the quantized part)
NQKV = JQ + 2 * HD  # 384
TT = S // P       # 16 token tiles
DT = D // P       # 16 d tiles
NB = 4            # t1 blocks
BN = S // NB      # 512
MAGIC = 12582912.0  # 1.5 * 2**23: (x + MAGIC) - MAGIC == round-half-even(x)
SM = HD ** -0.5   # 0.125
F32 = mybir.dt.float32
F32R = mybir.dt.float32r
BF16 = mybir.dt.bfloat16
FP16 = mybir.dt.float16
AF = mybir.ActivationFunctionType
ALU = mybir.AluOpType


def build_nc(debug_taps=False):
    nc = bacc.Bacc(target_bir_lowering=False, debug=False, num_devices=NCORES)
    xT = nc.declare_dram_parameter("xT", [D, S], F32R, isOutput=False)
    wqkv = nc.declare_dram_parameter("wqkv", [D, NQKV], F32R, isOutput=False)
    woT = nc.declare_dram_parameter("woT", [D, JQ], BF16, isOutput=False)
    tri = nc.declare_dram_parameter("tri", [P, P], BF16, isOutput=False)
    out_ext = nc.declare_dram_parameter("out", [JQ, S], F32, isOutput=True)

    taps = None
    if debug_taps:
        taps = {
            "qT_d": nc.declare_dram_parameter("qT_d", [P, 2, S], BF16, isOutput=True),
            "kT_d": nc.declare_dram_parameter("kT_d", [P, S], BF16, isOutput=True),
            "v_d": nc.declare_dram_parameter("v_d", [P, TT, HD + 1], BF16, isOutput=True),
            "ksr_d": nc.declare_dram_parameter("ksr_d", [P, TT], F32, isOutput=True),
            "qsrT_d": nc.declare_dram_parameter("qsrT_d", [97, S], F32, isOutput=True),
            "att_d": nc.declare_dram_parameter("att_d", [JQ, BN], BF16, isOutput=True),
            "attf_d": nc.declare_dram_parameter("attf_d", [NCORES * JQ, BN], BF16, isOutput=True),
        }
    with tile.TileContext(nc) as tc:
        with ExitStack() as ctx:
            _body(nc, tc, ctx, xT, wqkv, woT, tri, out_ext, taps)
    nc.finalize()
    return nc


def _body(nc, tc, ctx, xT, wqkv, woT, tri, out_ext, taps=None):
    # DRAM bounce buffers for the AllGather (one per t1 block)
    dram_pool = ctx.enter_context(tc.tile_pool(name="dram", bufs=1, space="DRAM"))
    att_shard = [
        dram_pool.tile([JQ, BN], BF16, name=f"att_shard{b}", tag=f"as{b}")
        for b in range(NB)
    ]
    att_full = [
        dram_pool.tile([NCORES * JQ, BN], BF16, addr_space="Shared",
                       name=f"att_full{b}", tag=f"af{b}")
        for b in range(NB)
    ]

    singles = ctx.enter_context(tc.tile_pool(name="singles", bufs=1))
    xpool = ctx.enter_context(tc.tile_pool(name="xpool", bufs=3))
    quant = ctx.enter_context(tc.tile_pool(name="quant", bufs=3))
    ei_pool = ctx.enter_context(tc.tile_pool(name="ei", bufs=4))
    p_pool = ctx.enter_context(tc.tile_pool(name="pp", bufs=4))
    bc_sb = ctx.enter_context(tc.tile_pool(name="bc_sb", bufs=3))
    an_sb = ctx.enter_context(tc.tile_pool(name="an_sb", bufs=3))
    orhs = ctx.enter_context(tc.tile_pool(name="orhs", bufs=4))
    osb = ctx.enter_context(tc.tile_pool(name="osb", bufs=3))
    # PSUM pools (8 banks of 2KB/partition total)
    ps_main = ctx.enter_context(tc.tile_pool(name="ps_main", bufs=2, space="PSUM"))
    ps_acc = ctx.enter_context(tc.tile_pool(name="ps_acc", bufs=4, space="PSUM"))
    ps_aux = ctx.enter_context(tc.tile_pool(name="ps_aux", bufs=2, space="PSUM"))

    # ---------------- persistent tiles ----------------
    wqkv_sb = singles.tile([P, DT, NQKV], F32R)
    _wsrc = wqkv.rearrange("(a p) n -> p a n", p=P)
    for c in range(4):
        nc.sync.dma_start(out=wqkv_sb[:, 4 * c:4 * c + 4, :],
                          in_=_wsrc[:, 4 * c:4 * c + 4, :])
    woT_sb = singles.tile([P, DT, JQ], BF16)
    nc.sync.dma_start(out=woT_sb, in_=woT.rearrange("(a p) n -> p a n", p=P))
    tri_sb = singles.tile([P, P], BF16)
    nc.sync.dma_start(out=tri_sb, in_=tri[:, :])
    id_fp16 = singles.tile([P, P], FP16)
    make_identity(nc, id_fp16)
    qT_sb = singles.tile([P, 2, S], FP16)   # dequantized q: [64*hh+hd, pair, t]
    kT_sb = singles.tile([P, S], FP16)      # dequantized k (incl sm), dup'd halves
    v_sb = singles.tile([P, TT, HD + 1], BF16)
    nc.vector.memset(v_sb, 1.0)             # col 64 stays 1.0 (sumexp trick)

    # ---------------- phase B: qkv projection + quantization ----------------
    for i in range(TT):
        xcol = xpool.tile([P, DT, P], F32R, tag="xcol")
        xsrc = xT[:, ts(i, P)].rearrange("(a p) m -> p a m", p=P)
        for c in range(4):
            nc.sync.dma_start(out=xcol[:, 4 * c:4 * c + 4, :],
                              in_=xsrc[:, 4 * c:4 * c + 4, :])
        qkv = ps_main.tile([P, NQKV], F32, tag="mm")
        for d in range(DT):
            nc.tensor.matmul(
                qkv, lhsT=xcol[:, d, :], rhs=wqkv_sb[:, d, :],
                start=(d == 0), stop=(d == DT - 1))
        # v -> bf16 (ones column at 64 preset)
        nc.vector.tensor_copy(v_sb[:, i, 0:HD], qkv[:, NQK:NQKV])
        # absmax over each head group of 64 (q heads 0-3, k group 4)
        amax = quant.tile([P, 5], F32, tag="amax")
        nc.vector.tensor_reduce(
            amax, qkv[:, 0:NQK].rearrange("p (g h) -> p g h", h=HD),
            axis=mybir.AxisListType.X, op=ALU.max, apply_absolute_value=True)
        amax_c = quant.tile([P, 5], F32, tag="amaxc")
        nc.vector.tensor_scalar_max(amax_c, amax, 1e-6)
        rec = quant.tile([P, 5], F32, tag="rec")
        nc.vector.reciprocal(rec, amax_c)
        scl = quant.tile([P, 5], F32, tag="scl")
        nc.vector.tensor_scalar_mul(scl, rec, 127.0)
        # dequant multipliers folded into the stored fp16 values:
        # q heads get amax/127, k gets amax*sm/127 (ints <=127 exact in fp16,
        # so only the final product rounds -- ~5e-4 relative)
        deq5 = quant.tile([P, 5], F32, tag="deq5")
        nc.vector.tensor_scalar_mul(deq5[:, 0:NHL], amax_c[:, 0:NHL], 1.0 / 127.0)
        nc.vector.tensor_scalar_mul(deq5[:, 4:5], amax_c[:, 4:5], SM / 127.0)
        # round-to-int via magic number: ACT does q*scl + MAGIC, then DVE does
        # (x - MAGIC) * deq -> fp16 in one tensor_scalar
        tmp = quant.tile([P, NQK], F32, tag="tmp")
        for h in range(5):
            nc.scalar.activation(
                out=tmp[:, ts(h, HD)], in_=qkv[:, ts(h, HD)],
                func=AF.Copy, scale=scl[:, h:h + 1], bias=MAGIC)
        qki = quant.tile([P, NQK], FP16, tag="qki")
        for h in range(5):
            nc.vector.tensor_scalar(
                qki[:, ts(h, HD)], tmp[:, ts(h, HD)], -MAGIC,
                deq5[:, h:h + 1], ALU.add, ALU.mult)
        # transposes: q halves -> qT pairs; k -> kT
        for half in range(2):
            tp = ps_aux.tile([P, P], FP16, tag="aux")
            nc.tensor.transpose(tp, qki[:, ts(half, P)], id_fp16)
            nc.vector.tensor_copy(qT_sb[:, half, ts(i, P)], tp)
        tpk = ps_aux.tile([HD, P], FP16, tag="aux")
        nc.tensor.transpose(tpk, qki[:, JQ:NQK], id_fp16)
        nc.vector.tensor_copy(kT_sb[0:HD, ts(i, P)], tpk)
    # duplicate k rows into partitions 64..127 (for row-group packed matmuls)
    nc.sync.dma_start(out=kT_sb[HD:P, :], in_=kT_sb[0:HD, :])

    if taps is not None:
        nc.sync.dma_start(out=taps["qT_d"][:, :, :], in_=qT_sb)
        nc.sync.dma_start(out=taps["kT_d"][:, :], in_=kT_sb)
        nc.sync.dma_start(out=taps["v_d"][:, :, :], in_=v_sb)
        nc.sync.dma_start(out=taps["ksr_d"][:, :], in_=ksr_sb)
        nc.sync.dma_start(out=taps["qsrT_d"][:, :], in_=qsrT_sb)

    # ---------------- phase C/D: attention blocks + AG + o_proj ----------------
    for b in range(NB):
        na = 4 * (b + 1)
        for pair in range(2):
            heads = (2 * pair, 2 * pair + 1)
            # broadcast qsr rows across 128 partitions via K=1 matmul
            atps = [ps_acc.tile([HD + 1, BN], F32, tag="acc", name=f"at{b}_{pair}_{hh}") for hh in range(2)]
            for a in range(na):
                arel = a - 4 * b
                off = max(0, arel) * P
                n_sub = BN - off
                for hh, h in enumerate(heads):
                    rows = slice(HD * hh, HD * hh + HD)
                    sc = ps_main.tile([P, BN], F32, tag="mm")
                    nc.tensor.matmul(
                        sc[:, off:], lhsT=kT_sb[rows, ts(a, P)],
                        rhs=qT_sb[rows, pair, ds(b * BN + off, n_sub)],
                        start=True, stop=True)
                    pt = p_pool.tile([P, BN], BF16, tag="pt")
                    nc.scalar.activation(
                        out=pt[:, off:], in_=sc[:, off:], func=AF.Exp)
                    if arel >= 0:
                        nc.vector.tensor_mul(
                            pt[:, off:off + P], pt[:, off:off + P], tri_sb)
                    nc.tensor.matmul(
                        atps[hh][:, off:], lhsT=v_sb[:, a, :], rhs=pt[:, off:],
                        start=(a == 0), stop=(a == na - 1))
            # normalize by sumexp (row 64) and emit bf16 shard
            for hh, h in enumerate(heads):
                se = bc_sb.tile([65, BN], F32, tag="se")
                nc.vector.tensor_copy(se[HD:HD + 1, :], atps[hh][HD:HD + 1, :])
                se0 = bc_sb.tile([1, BN], F32, tag="se0")
                nc.sync.dma_start(out=se0[0:1, :], in_=se[HD:HD + 1, :])
                rcp0 = bc_sb.tile([1, BN], F32, tag="rcp0")
                nc.vector.reciprocal_approx_fast(rcp0, se0)
                rbs = bc_sb.tile([HD, BN], F32, tag="rbs")
                nc.gpsimd.partition_broadcast(
                    rbs, rcp0[0:1, :], channels=HD)
                ans = an_sb.tile([HD, BN], BF16, tag="ans")
                nc.vector.tensor_mul(ans, atps[hh][0:HD, :], rbs)
                nc.sync.dma_start(out=att_shard[b][ts(h, HD), :], in_=ans)
        nc.gpsimd.collective_compute(
            "AllGather", ALU.bypass,
            replica_groups=[list(range(NCORES))],
            ins=[att_shard[b][:, :]], outs=[att_full[b][:, :]])
        if taps is not None and b == 0:
            nc.sync.dma_start(out=taps["att_d"][:, :], in_=att_shard[0][:, :])
            nc.sync.dma_start(out=taps["attf_d"][:, :], in_=att_full[0][:, :])
        # o_proj for this t1 block (column shard of Wo -> no reduce needed)
        oph = [ps_acc.tile([P, BN], F32, tag="acc", name=f"op{b}_{m}") for m in range(2)]
        for j in range(DT):
            rt = orhs.tile([P, BN], BF16, tag="rt")
            nc.sync.dma_start(out=rt, in_=att_full[b][ts(j, P), :])
            for m in range(2):
                nc.tensor.matmul(
                    oph[m], lhsT=woT_sb[:, j, ts(m, P)], rhs=rt,
                    start=(j == 0), stop=(j == DT - 1))
        for m in range(2):
            ot = osb.tile([P, BN], F32, tag="ot")
            nc.vector.tensor_copy(ot, oph[m])
            nc.sync.dma_start(out=out_ext[ts(m, P), ts(b, BN)], in_=ot)


# ---------------- host side ----------------

def prep_in_maps(x, Wq, Wk, Wv, Wo):
    bf = ml_dtypes.bfloat16
    xTh = np.ascontiguousarray(x.reshape(S, D).T.astype(np.float32))
    tri_h = np.ascontiguousarray(
        (np.arange(P)[:, None] <= np.arange(P)[None, :]).astype(bf))
    in_maps = []
    for c in range(NCORES):
        wq = Wq[c * JQ:(c + 1) * JQ, :].T
        wk = Wk[c * HD:(c + 1) * HD, :].T
        wv = Wv[c * HD:(c + 1) * HD, :].T
        wqkv_h = np.ascontiguousarray(
            np.concatenate([wq, wk, wv], axis=1).astype(np.float32))
        woT_h = np.ascontiguousarray(
            Wo[c * JQ:(c + 1) * JQ, :].T.astype(bf))
        in_maps.append({"xT": xTh, "wqkv": wqkv_h, "woT": woT_h, "tri": tri_h})
    return in_maps


def unshard(results):
    out = np.empty((S, D), dtype=np.float32)
    for c in range(NCORES):
        out[:, c * JQ:(c + 1) * JQ] = results[c]["out"].T
    return out.reshape(1, S, D)


def kernel(x, Wq, Wk, Wv, Wo):
    from concourse.bass_utils import run_bass_kernel_spmd
    nc = build_nc()
    in_maps = prep_in_maps(x, Wq, Wk, Wv, Wo)
    res = run_bass_kernel_spmd(nc, in_maps, core_ids=list(range(NCORES)))
    return unshard(res.results)


# revision 16
# speedup vs baseline: 1.3982x; 1.2771x over previous
"""Trainium2 Bass kernel for GQA attention with int8-quantized QK^T (8-core SPMD).

Reference (per-core shard c of 8):
  q = x @ Wq.T          -> heads [4c..4c+4), computed as q[t, 256]
  k = x @ Wk.T, v = x @ Wv.T  -> kv-head c, [t, 64] each
  per-token-per-head int8 absmax quantization of q, k (exact emulation:
  integer values live in bf16 -- integers <= 127 are exact in bf16, and the
  i8xi8 -> i32 dot over 64 terms (<2^24) is exact in f32 PSUM accumulate)
  scoresT[t2, t1] = k_i8.T @ q_i8 ; dequant = scoresT * ksr[t2] * qsr[t1]
  p = exp(dequant) (no max-subtraction; |arg| <= ~58 is safe in f32)
  attT[hd, t1] = v_aug.T @ p  with ones column -> row 64 = sumexp
  normalize, AllGather heads across cores, out_c = WoT_c.T @ attT_full
  (o_proj is column-sharded -> host concatenates; no AllReduce needed)

Layouts are transposed throughout ([feature, token]) so softmax runs along
the free axis of nothing -- the only transposes are the 128-wide PE
transposes of q_i8/k_i8/qsr after quantization (natural layout is needed
for the per-token absmax along the free axis).
"""

import numpy as np
import ml_dtypes
from contextlib import ExitStack

import concourse.bass as bass
import concourse.mybir as mybir
import concourse.tile as tile
from concourse import bacc
from concourse.bass import ts, ds
from concourse.masks import make_identity

NCORES = 8
P = 128
S = 2048          # tokens
D = 2048          # model dim
HD = 64           # head dim
NHL = 4           # q heads per core
JQ = NHL * HD     # 256 (q cols per core)
NQK = JQ + HD     # 320 (q + k cols, the quantized part)
NQKV = JQ + 2 * HD  # 384
TT = S // P       # 16 token tiles
DT = D // P       # 16 d tiles
NB = 4            # t1 blocks
BN = S // NB      # 512
MAGIC = 12582912.0  # 1.5 * 2**23: (x + MAGIC) - MAGIC == round-half-even(x)
SM = HD ** -0.5   # 0.125
F32 = mybir.dt.float32
F32R = mybir.dt.float32r
BF16 = mybir.dt.bfloat16
FP16 = mybir.dt.float16
AF = mybir.ActivationFunctionType
ALU = mybir.AluOpType


def build_nc(debug_taps=False):
    nc = bacc.Bacc(target_bir_lowering=False, debug=False, num_devices=NCORES)
    xT = nc.declare_dram_parameter("xT", [D, S], F32R, isOutput=False)
    wqkv = nc.declare_dram_parameter("wqkv", [D, NQKV], F32R, isOutput=False)
    woT = nc.declare_dram_parameter("woT", [D, JQ], BF16, isOutput=False)
    tri = nc.declare_dram_parameter("tri", [P, P], BF16, isOutput=False)
    out_ext = nc.declare_dram_parameter("out", [JQ, S], F32, isOutput=True)

    taps = None
    if debug_taps:
        taps = {
            "qT_d": nc.declare_dram_parameter("qT_d", [P, 2, S], BF16, isOutput=True),
            "kT_d": nc.declare_dram_parameter("kT_d", [P, S], BF16, isOutput=True),
            "v_d": nc.declare_dram_parameter("v_d", [P, TT, HD + 1], BF16, isOutput=True),
            "ksr_d": nc.declare_dram_parameter("ksr_d", [P, TT], F32, isOutput=True),
            "qsrT_d": nc.declare_dram_parameter("qsrT_d", [97, S], F32, isOutput=True),
            "att_d": nc.declare_dram_parameter("att_d", [JQ, BN], BF16, isOutput=True),
            "attf_d": nc.declare_dram_parameter("attf_d", [NCORES * JQ, BN], BF16, isOutput=True),
        }
    with tile.TileContext(nc) as tc:
        with ExitStack() as ctx:
            _body(nc, tc, ctx, xT, wqkv, woT, tri, out_ext, taps)
    nc.finalize()
    return nc


def _body(nc, tc, ctx, xT, wqkv, woT, tri, out_ext, taps=None):
    # DRAM bounce buffers for the AllGather (one per t1 block)
    dram_pool = ctx.enter_context(tc.tile_pool(name="dram", bufs=1, space="DRAM"))
    att_shard = [
        dram_pool.tile([JQ, BN], BF16, name=f"att_shard{b}", tag=f"as{b}")
        for b in range(NB)
    ]
    att_full = [
        dram_pool.tile([NCORES * JQ, BN], BF16, addr_space="Shared",
                       name=f"att_full{b}", tag=f"af{b}")
        for b in range(NB)
    ]

    singles = ctx.enter_context(tc.tile_pool(name="singles", bufs=1))
    xpool = ctx.enter_context(tc.tile_pool(name="xpool", bufs=3))
    quant = ctx.enter_context(tc.tile_pool(name="quant", bufs=3))
    ei_pool = ctx.enter_context(tc.tile_pool(name="ei", bufs=4))
    p_pool = ctx.enter_context(tc.tile_pool(name="pp", bufs=4))
    bc_sb = ctx.enter_context(tc.tile_pool(name="bc_sb", bufs=3))
    an_sb = ctx.enter_context(tc.tile_pool(name="an_sb", bufs=3))
    orhs = ctx.enter_context(tc.tile_pool(name="orhs", bufs=4))
    osb = ctx.enter_context(tc.tile_pool(name="osb", bufs=3))
    # PSUM pools (8 banks of 2KB/partition total)
    ps_main = ctx.enter_context(tc.tile_pool(name="ps_main", bufs=2, space="PSUM"))
    ps_acc = ctx.enter_context(tc.tile_pool(name="ps_acc", bufs=4, space="PSUM"))
    ps_aux = ctx.enter_context(tc.tile_pool(name="ps_aux", bufs=2, space="PSUM"))

    # ---------------- persistent tiles ----------------
    wqkv_sb = singles.tile([P, DT, NQKV], F32R)
    _wsrc = wqkv.rearrange("(a p) n -> p a n", p=P)
    for c in range(4):
        nc.sync.dma_start(out=wqkv_sb[:, 4 * c:4 * c + 4, :],
                          in_=_wsrc[:, 4 * c:4 * c + 4, :])
    woT_sb = singles.tile([P, DT, JQ], BF16)
    nc.sync.dma_start(out=woT_sb, in_=woT.rearrange("(a p) n -> p a n", p=P))
    tri_sb = singles.tile([P, P], BF16)
    nc.sync.dma_start(out=tri_sb, in_=tri[:, :])
    id_fp16 = singles.tile([P, P], FP16)
    make_identity(nc, id_fp16)
    qT_sb = singles.tile([P, 2, S], FP16)   # dequantized q: [64*hh+hd, pair, t]
    kT_sb = singles.tile([P, S], FP16)      # dequantized k (incl sm), dup'd halves
    v_sb = singles.tile([P, TT, HD + 1], BF16)
    nc.vector.memset(v_sb, 1.0)             # col 64 stays 1.0 (sumexp trick)

    # ---------------- phase B: qkv projection + quantization ----------------
    for i in range(TT):
        xcol = xpool.tile([P, DT, P], F32R, tag="xcol")
        xsrc = xT[:, ts(i, P)].rearrange("(a p) m -> p a m", p=P)
        for c in range(4):
            nc.sync.dma_start(out=xcol[:, 4 * c:4 * c + 4, :],
                              in_=xsrc[:, 4 * c:4 * c + 4, :])
        qkv = ps_main.tile([P, NQKV], F32, tag="mm")
        for d in range(DT):
            nc.tensor.matmul(
                qkv, lhsT=xcol[:, d, :], rhs=wqkv_sb[:, d, :],
                start=(d == 0), stop=(d == DT - 1))
        # v -> bf16 (ones column at 64 preset)
        nc.vector.tensor_copy(v_sb[:, i, 0:HD], qkv[:, NQK:NQKV])
        # absmax over each head group of 64 (q heads 0-3, k group 4)
        amax = quant.tile([P, 5], F32, tag="amax")
        nc.vector.tensor_reduce(
            amax, qkv[:, 0:NQK].rearrange("p (g h) -> p g h", h=HD),
            axis=mybir.AxisListType.X, op=ALU.max, apply_absolute_value=True)
        amax_c = quant.tile([P, 5], F32, tag="amaxc")
        nc.vector.tensor_scalar_max(amax_c, amax, 1e-6)
        rec = quant.tile([P, 5], F32, tag="rec")
        nc.vector.reciprocal(rec, amax_c)
        scl = quant.tile([P, 5], F32, tag="scl")
        nc.vector.tensor_scalar_mul(scl, rec, 127.0)
        # dequant multipliers folded into the stored fp16 values:
        # q heads get amax/127, k gets amax*sm/127 (ints <=127 exact in fp16,
        # so only the final product rounds -- ~5e-4 relative)
        deq5 = quant.tile([P, 5], F32, tag="deq5")
        nc.vector.tensor_scalar_mul(deq5[:, 0:NHL], amax_c[:, 0:NHL], 1.0 / 127.0)
        nc.vector.tensor_scalar_mul(deq5[:, 4:5], amax_c[:, 4:5], SM / 127.0)
        # round-to-int via magic number: ACT does q*scl + MAGIC, then DVE does
        # (x - MAGIC) * deq -> fp16 in one tensor_scalar
        tmp = quant.tile([P, NQK], F32, tag="tmp")
        for h in range(5):
            nc.scalar.activation(
                out=tmp[:, ts(h, HD)], in_=qkv[:, ts(h, HD)],
                func=AF.Copy, scale=scl[:, h:h + 1], bias=MAGIC)
        qki = quant.tile([P, NQK], FP16, tag="qki")
        for h in range(5):
            nc.vector.tensor_scalar(
                qki[:, ts(h, HD)], tmp[:, ts(h, HD)], -MAGIC,
                deq5[:, h:h + 1], ALU.add, ALU.mult)
        # transposes: q halves -> qT pairs; k -> kT
        for half in range(2):
            tp = ps_aux.tile([P, P], FP16, tag="aux")
            nc.tensor.transpose(tp, qki[:, ts(half, P)], id_fp16)
            nc.vector.tensor_copy(qT_sb[:, half, ts(i, P)], tp)
        tpk = ps_aux.tile([HD, P], FP16, tag="aux")
        nc.tensor.transpose(tpk, qki[:, JQ:NQK], id_fp16)
        nc.vector.tensor_copy(kT_sb[0:HD, ts(i, P)], tpk)
    # duplicate k rows into partitions 64..127 (for row-group packed matmuls)
    nc.sync.dma_start(out=kT_sb[HD:P, :], in_=kT_sb[0:HD, :])

    if taps is not None:
        nc.sync.dma_start(out=taps["qT_d"][:, :, :], in_=qT_sb)
        nc.sync.dma_start(out=taps["kT_d"][:, :], in_=kT_sb)
        nc.sync.dma_start(out=taps["v_d"][:, :, :], in_=v_sb)

    # ---------------- phase C/D: attention blocks + AG + o_proj ----------------
    for b in range(NB):
        na = 4 * (b + 1)
        for pair in range(2):
            heads = (2 * pair, 2 * pair + 1)
            # broadcast qsr rows across 128 partitions via K=1 matmul
            atps = [ps_acc.tile([HD + 1, BN], F32, tag="acc", name=f"at{b}_{pair}_{hh}") for hh in range(2)]
            for a in range(na):
                arel = a - 4 * b
                off = max(0, arel) * P
                n_sub = BN - off
                for hh, h in enumerate(heads):
                    rows = slice(HD * hh, HD * hh + HD)
                    sc = ps_main.tile([P, BN], F32, tag="mm")
                    nc.tensor.matmul(
                        sc[:, off:], lhsT=kT_sb[rows, ts(a, P)],
                        rhs=qT_sb[rows, pair, ds(b * BN + off, n_sub)],
                        start=True, stop=True)
                    pt = p_pool.tile([P, BN], BF16, tag="pt")
                    nc.scalar.activation(
                        out=pt[:, off:], in_=sc[:, off:], func=AF.Exp)
                    if arel >= 0:
                        nc.vector.tensor_mul(
                            pt[:, off:off + P], pt[:, off:off + P], tri_sb)
                    nc.tensor.matmul(
                        atps[hh][:, off:], lhsT=v_sb[:, a, :], rhs=pt[:, off:],
                        start=(a == 0), stop=(a == na - 1))
            # normalize by sumexp (row 64) and emit bf16 shard
            for hh, h in enumerate(heads):
                se = bc_sb.tile([65, BN], F32, tag="se")
                nc.vector.tensor_copy(se[HD:HD + 1, :], atps[hh][HD:HD + 1, :])
                se0 = bc_sb.tile([1, BN], F32, tag="se0")
                nc.sync.dma_start(out=se0[0:1, :], in_=se[HD:HD + 1, :])
                rcp0 = bc_sb.tile([1, BN], F32, tag="rcp0")
                nc.vector.reciprocal_approx_fast(rcp0, se0)
                rbs = bc_sb.tile([HD, BN], F32, tag="rbs")
                nc.gpsimd.partition_broadcast(
                    rbs, rcp0[0:1, :], channels=HD)
                ans = an_sb.tile([HD, BN], BF16, tag="ans")
                nc.vector.tensor_mul(ans, atps[hh][0:HD, :], rbs)
                nc.sync.dma_start(out=att_shard[b][ts(h, HD), :], in_=ans)
        nc.gpsimd.collective_compute(
            "AllGather", ALU.bypass,
            replica_groups=[list(range(NCORES))],
            ins=[att_shard[b][:, :]], outs=[att_full[b][:, :]])
        if taps is not None and b == 0:
            nc.sync.dma_start(out=taps["att_d"][:, :], in_=att_shard[0][:, :])
            nc.sync.dma_start(out=taps["attf_d"][:, :], in_=att_full[0][:, :])

    # ---------------- phase D: o_proj, emitted after ALL attention blocks so
    # the PE instruction stream never stalls on an in-flight AllGather ----------
    for b in range(NB):
        oph = [ps_acc.tile([P, BN], F32, tag="acc", name=f"op{b}_{m}") for m in range(2)]
        for j in range(DT):
            rt = orhs.tile([P, BN], BF16, tag="rt")
            nc.sync.dma_start(out=rt, in_=att_full[b][ts(j, P), :])
            for m in range(2):
                nc.tensor.matmul(
                    oph[m], lhsT=woT_sb[:, j, ts(m, P)], rhs=rt,
                    start=(j == 0), stop=(j == DT - 1))
        for m in range(2):
            ot = osb.tile([P, BN], F32, tag="ot")
            nc.vector.tensor_copy(ot, oph[m])
            nc.sync.dma_start(out=out_ext[ts(m, P), ts(b, BN)], in_=ot)


# ---------------- host side ----------------

def prep_in_maps(x, Wq, Wk, Wv, Wo):
    bf = ml_dtypes.bfloat16
    xTh = np.ascontiguousarray(x.reshape(S, D).T.astype(np.float32))
    tri_h = np.ascontiguousarray(
        (np.arange(P)[:, None] <= np.arange(P)[None, :]).astype(bf))
    in_maps = []
    for c in range(NCORES):
        wq = Wq[c * JQ:(c + 1) * JQ, :].T
        wk = Wk[c * HD:(c + 1) * HD, :].T
        wv = Wv[c * HD:(c + 1) * HD, :].T
        wqkv_h = np.ascontiguousarray(
            np.concatenate([wq, wk, wv], axis=1).astype(np.float32))
        woT_h = np.ascontiguousarray(
            Wo[c * JQ:(c + 1) * JQ, :].T.astype(bf))
        in_maps.append({"xT": xTh, "wqkv": wqkv_h, "woT": woT_h, "tri": tri_h})
    return in_maps


def unshard(results):
    out = np.empty((S, D), dtype=np.float32)
    for c in range(NCORES):
        out[:, c * JQ:(c + 1) * JQ] = results[c]["out"].T
    return out.reshape(1, S, D)


def kernel(x, Wq, Wk, Wv, Wo):
    from concourse.bass_utils import run_bass_kernel_spmd
    nc = build_nc()
    in_maps = prep_in_maps(x, Wq, Wk, Wv, Wo)
    res = run_bass_kernel_spmd(nc, in_maps, core_ids=list(range(NCORES)))
    return unshard(res.results)


# revision 17
# speedup vs baseline: 1.4154x; 1.0123x over previous
"""Trainium2 Bass kernel for GQA attention with int8-quantized QK^T (8-core SPMD).

Reference (per-core shard c of 8):
  q = x @ Wq.T          -> heads [4c..4c+4), computed as q[t, 256]
  k = x @ Wk.T, v = x @ Wv.T  -> kv-head c, [t, 64] each
  per-token-per-head int8 absmax quantization of q, k (exact emulation:
  integer values live in bf16 -- integers <= 127 are exact in bf16, and the
  i8xi8 -> i32 dot over 64 terms (<2^24) is exact in f32 PSUM accumulate)
  scoresT[t2, t1] = k_i8.T @ q_i8 ; dequant = scoresT * ksr[t2] * qsr[t1]
  p = exp(dequant) (no max-subtraction; |arg| <= ~58 is safe in f32)
  attT[hd, t1] = v_aug.T @ p  with ones column -> row 64 = sumexp
  normalize, AllGather heads across cores, out_c = WoT_c.T @ attT_full
  (o_proj is column-sharded -> host concatenates; no AllReduce needed)

Layouts are transposed throughout ([feature, token]) so softmax runs along
the free axis of nothing -- the only transposes are the 128-wide PE
transposes of q_i8/k_i8/qsr after quantization (natural layout is needed
for the per-token absmax along the free axis).
"""

import numpy as np
import ml_dtypes
from contextlib import ExitStack

import concourse.bass as bass
import concourse.mybir as mybir
import concourse.tile as tile
from concourse import bacc
from concourse.bass import ts, ds
from concourse.masks import make_identity

NCORES = 8
P = 128
S = 2048          # tokens
D = 2048          # model dim
HD = 64           # head dim
NHL = 4           # q heads per core
JQ = NHL * HD     # 256 (q cols per core)
NQK = JQ + HD     # 320 (q + k cols, the quantized part)
NQKV = JQ + 2 * HD  # 384
TT = S // P       # 16 token tiles
DT = D // P       # 16 d tiles
NB = 4            # t1 blocks
BN = S // NB      # 512
MAGIC = 12582912.0  # 1.5 * 2**23: (x + MAGIC) - MAGIC == round-half-even(x)
SM = HD ** -0.5   # 0.125
F32 = mybir.dt.float32
F32R = mybir.dt.float32r
BF16 = mybir.dt.bfloat16
FP16 = mybir.dt.float16
AF = mybir.ActivationFunctionType
ALU = mybir.AluOpType


def build_nc(debug_taps=False):
    nc = bacc.Bacc(target_bir_lowering=False, debug=False, num_devices=NCORES)
    xT = nc.declare_dram_parameter("xT", [D, S], F32R, isOutput=False)
    wqkv = nc.declare_dram_parameter("wqkv", [D, NQKV], F32R, isOutput=False)
    woT = nc.declare_dram_parameter("woT", [D, JQ], BF16, isOutput=False)
    tri = nc.declare_dram_parameter("tri", [P, P], BF16, isOutput=False)
    out_ext = nc.declare_dram_parameter("out", [JQ, S], F32, isOutput=True)

    taps = None
    if debug_taps:
        taps = {
            "qT_d": nc.declare_dram_parameter("qT_d", [P, 2, S], BF16, isOutput=True),
            "kT_d": nc.declare_dram_parameter("kT_d", [P, S], BF16, isOutput=True),
            "v_d": nc.declare_dram_parameter("v_d", [P, TT, HD + 1], BF16, isOutput=True),
            "ksr_d": nc.declare_dram_parameter("ksr_d", [P, TT], F32, isOutput=True),
            "qsrT_d": nc.declare_dram_parameter("qsrT_d", [97, S], F32, isOutput=True),
            "att_d": nc.declare_dram_parameter("att_d", [JQ, BN], BF16, isOutput=True),
            "attf_d": nc.declare_dram_parameter("attf_d", [NCORES * JQ, BN], BF16, isOutput=True),
        }
    with tile.TileContext(nc) as tc:
        with ExitStack() as ctx:
            _body(nc, tc, ctx, xT, wqkv, woT, tri, out_ext, taps)
    nc.finalize()
    return nc


def _body(nc, tc, ctx, xT, wqkv, woT, tri, out_ext, taps=None):
    # DRAM bounce buffers for the AllGather (one per t1 block)
    dram_pool = ctx.enter_context(tc.tile_pool(name="dram", bufs=1, space="DRAM"))
    att_shard = [
        dram_pool.tile([JQ, BN], BF16, name=f"att_shard{b}", tag=f"as{b}")
        for b in range(NB)
    ]
    att_full = [
        dram_pool.tile([NCORES * JQ, BN], BF16, addr_space="Shared",
                       name=f"att_full{b}", tag=f"af{b}")
        for b in range(NB)
    ]

    singles = ctx.enter_context(tc.tile_pool(name="singles", bufs=1))
    xpool = ctx.enter_context(tc.tile_pool(name="xpool", bufs=3))
    quant = ctx.enter_context(tc.tile_pool(name="quant", bufs=3))
    ei_pool = ctx.enter_context(tc.tile_pool(name="ei", bufs=4))
    p_pool = ctx.enter_context(tc.tile_pool(name="pp", bufs=4))
    bc_sb = ctx.enter_context(tc.tile_pool(name="bc_sb", bufs=3))
    an_sb = ctx.enter_context(tc.tile_pool(name="an_sb", bufs=3))
    orhs = ctx.enter_context(tc.tile_pool(name="orhs", bufs=8))
    osb = ctx.enter_context(tc.tile_pool(name="osb", bufs=3))
    # PSUM pools (8 banks of 2KB/partition total)
    ps_main = ctx.enter_context(tc.tile_pool(name="ps_main", bufs=3, space="PSUM"))
    ps_at = ctx.enter_context(tc.tile_pool(name="ps_at", bufs=3, space="PSUM"))
    ps_aux = ctx.enter_context(tc.tile_pool(name="ps_aux", bufs=2, space="PSUM"))

    # ---------------- persistent tiles ----------------
    wqkv_sb = singles.tile([P, DT, NQKV], F32R)
    _wsrc = wqkv.rearrange("(a p) n -> p a n", p=P)
    for c in range(DT):
        nc.sync.dma_start(out=wqkv_sb[:, c:c + 1, :], in_=_wsrc[:, c:c + 1, :])
    woT_sb = singles.tile([P, DT, JQ], BF16)
    tri_sb = singles.tile([P, P], BF16)
    nc.sync.dma_start(out=tri_sb, in_=tri[:, :])
    id_fp16 = singles.tile([P, P], FP16)
    make_identity(nc, id_fp16)
    qT_sb = singles.tile([P, 2, S], FP16)   # dequantized q: [64*hh+hd, pair, t]
    kT_sb = singles.tile([P, S], FP16)      # dequantized k (incl sm), dup'd halves
    v_sb = singles.tile([P, TT, HD + 1], BF16)
    nc.vector.memset(v_sb, 1.0)             # col 64 stays 1.0 (sumexp trick)

    # ---------------- phase B: qkv projection + quantization ----------------
    for i in range(TT):
        xcol = xpool.tile([P, DT, P], F32R, tag="xcol")
        xsrc = xT[:, ts(i, P)].rearrange("(a p) m -> p a m", p=P)
        for c in range(4):
            nc.sync.dma_start(out=xcol[:, 4 * c:4 * c + 4, :],
                              in_=xsrc[:, 4 * c:4 * c + 4, :])
        qkv = ps_main.tile([P, NQKV], F32, tag="mm")
        for d in range(DT):
            nc.tensor.matmul(
                qkv, lhsT=xcol[:, d, :], rhs=wqkv_sb[:, d, :],
                start=(d == 0), stop=(d == DT - 1))
        # v -> bf16 (ones column at 64 preset)
        nc.vector.tensor_copy(v_sb[:, i, 0:HD], qkv[:, NQK:NQKV])
        # absmax over each head group of 64 (q heads 0-3, k group 4)
        amax = quant.tile([P, 5], F32, tag="amax")
        nc.vector.tensor_reduce(
            amax, qkv[:, 0:NQK].rearrange("p (g h) -> p g h", h=HD),
            axis=mybir.AxisListType.X, op=ALU.max, apply_absolute_value=True)
        amax_c = quant.tile([P, 5], F32, tag="amaxc")
        nc.vector.tensor_scalar_max(amax_c, amax, 1e-6)
        rec = quant.tile([P, 5], F32, tag="rec")
        nc.vector.reciprocal(rec, amax_c)
        scl = quant.tile([P, 5], F32, tag="scl")
        nc.vector.tensor_scalar_mul(scl, rec, 127.0)
        # dequant multipliers folded into the stored fp16 values:
        # q heads get amax/127, k gets amax*sm/127 (ints <=127 exact in fp16,
        # so only the final product rounds -- ~5e-4 relative)
        deq5 = quant.tile([P, 5], F32, tag="deq5")
        nc.vector.tensor_scalar_mul(deq5[:, 0:NHL], amax_c[:, 0:NHL], 1.0 / 127.0)
        nc.vector.tensor_scalar_mul(deq5[:, 4:5], amax_c[:, 4:5], SM / 127.0)
        # round-to-int via magic number: ACT does q*scl + MAGIC, then DVE does
        # (x - MAGIC) * deq -> fp16 in one tensor_scalar
        tmp = quant.tile([P, NQK], F32, tag="tmp")
        for h in range(5):
            nc.scalar.activation(
                out=tmp[:, ts(h, HD)], in_=qkv[:, ts(h, HD)],
                func=AF.Copy, scale=scl[:, h:h + 1], bias=MAGIC)
        qki = quant.tile([P, NQK], FP16, tag="qki")
        for h in range(5):
            nc.vector.tensor_scalar(
                qki[:, ts(h, HD)], tmp[:, ts(h, HD)], -MAGIC,
                deq5[:, h:h + 1], ALU.add, ALU.mult)
        # transposes: q halves -> qT pairs; k -> kT
        for half in range(2):
            tp = ps_aux.tile([P, P], FP16, tag="aux")
            nc.tensor.transpose(tp, qki[:, ts(half, P)], id_fp16)
            nc.vector.tensor_copy(qT_sb[:, half, ts(i, P)], tp)
        tpk = ps_aux.tile([HD, P], FP16, tag="aux")
        nc.tensor.transpose(tpk, qki[:, JQ:NQK], id_fp16)
        nc.vector.tensor_copy(kT_sb[0:HD, ts(i, P)], tpk)
    # duplicate k rows into partitions 64..127 (for row-group packed matmuls)
    nc.sync.dma_start(out=kT_sb[HD:P, :], in_=kT_sb[0:HD, :])

    if taps is not None:
        nc.sync.dma_start(out=taps["qT_d"][:, :, :], in_=qT_sb)
        nc.sync.dma_start(out=taps["kT_d"][:, :], in_=kT_sb)
        nc.sync.dma_start(out=taps["v_d"][:, :, :], in_=v_sb)

    # ---------------- phase C/D: attention blocks + AG + o_proj ----------------
    for b in range(NB):
        na = 4 * (b + 1)
        for pair in range(2):
            heads = (2 * pair, 2 * pair + 1)
            # broadcast qsr rows across 128 partitions via K=1 matmul
            atps = [ps_at.tile([HD + 1, BN], F32, tag="at", name=f"at{b}_{pair}_{hh}") for hh in range(2)]
            for a in range(na):
                arel = a - 4 * b
                off = max(0, arel) * P
                n_sub = BN - off
                for hh, h in enumerate(heads):
                    rows = slice(HD * hh, HD * hh + HD)
                    sc = ps_main.tile([P, BN], F32, tag="mm")
                    nc.tensor.matmul(
                        sc[:, off:], lhsT=kT_sb[rows, ts(a, P)],
                        rhs=qT_sb[rows, pair, ds(b * BN + off, n_sub)],
                        start=True, stop=True)
                    pt = p_pool.tile([P, BN], BF16, tag="pt")
                    nc.scalar.activation(
                        out=pt[:, off:], in_=sc[:, off:], func=AF.Exp)
                    if arel >= 0:
                        nc.vector.tensor_mul(
                            pt[:, off:off + P], pt[:, off:off + P], tri_sb)
                    nc.tensor.matmul(
                        atps[hh][:, off:], lhsT=v_sb[:, a, :], rhs=pt[:, off:],
                        start=(a == 0), stop=(a == na - 1))
            # normalize by sumexp (row 64) and emit bf16 shard
            for hh, h in enumerate(heads):
                se = bc_sb.tile([65, BN], F32, tag="se")
                nc.vector.tensor_copy(se[HD:HD + 1, :], atps[hh][HD:HD + 1, :])
                se0 = bc_sb.tile([1, BN], F32, tag="se0")
                nc.sync.dma_start(out=se0[0:1, :], in_=se[HD:HD + 1, :])
                rcp0 = bc_sb.tile([1, BN], F32, tag="rcp0")
                nc.vector.reciprocal_approx_fast(rcp0, se0)
                rbs = bc_sb.tile([HD, BN], F32, tag="rbs")
                nc.gpsimd.partition_broadcast(
                    rbs, rcp0[0:1, :], channels=HD)
                ans = an_sb.tile([HD, BN], BF16, tag="ans")
                nc.vector.tensor_mul(ans, atps[hh][0:HD, :], rbs)
                nc.sync.dma_start(out=att_shard[b][ts(h, HD), :], in_=ans)
        nc.gpsimd.collective_compute(
            "AllGather", ALU.bypass,
            replica_groups=[list(range(NCORES))],
            ins=[att_shard[b][:, :]], outs=[att_full[b][:, :]])
        if taps is not None and b == 0:
            nc.sync.dma_start(out=taps["att_d"][:, :], in_=att_shard[0][:, :])
            nc.sync.dma_start(out=taps["attf_d"][:, :], in_=att_full[0][:, :])
        if b == 0:
            nc.sync.dma_start(out=woT_sb, in_=woT.rearrange("(a p) n -> p a n", p=P))
        if b >= 1:
            _oproj(nc, b - 1, ps_aux, orhs, osb, woT_sb, att_full, out_ext)
    _oproj(nc, NB - 1, ps_aux, orhs, osb, woT_sb, att_full, out_ext)


def _oproj(nc, b, ps_aux, orhs, osb, woT_sb, att_full, out_ext):
    oph = [ps_aux.tile([P, BN], F32, tag="aux", name=f"op{b}_{m}") for m in range(2)]
    for j in range(DT):
        rt = orhs.tile([P, BN], BF16, tag="rt", name=f"rt{b}_{j}")
        nc.sync.dma_start(out=rt, in_=att_full[b][ts(j, P), :])
        for m in range(2):
            nc.tensor.matmul(
                oph[m], lhsT=woT_sb[:, j, ts(m, P)], rhs=rt,
                start=(j == 0), stop=(j == DT - 1))
    for m in range(2):
        ot = osb.tile([P, BN], F32, tag="ot", name=f"ot{b}_{m}")
        nc.vector.tensor_copy(ot, oph[m])
        nc.sync.dma_start(out=out_ext[ts(m, P), ts(b, BN)], in_=ot)


# ---------------- host side ----------------

def prep_in_maps(x, Wq, Wk, Wv, Wo):
    bf = ml_dtypes.bfloat16
    xTh = np.ascontiguousarray(x.reshape(S, D).T.astype(np.float32))
    tri_h = np.ascontiguousarray(
        (np.arange(P)[:, None] <= np.arange(P)[None, :]).astype(bf))
    in_maps = []
    for c in range(NCORES):
        wq = Wq[c * JQ:(c + 1) * JQ, :].T
        wk = Wk[c * HD:(c + 1) * HD, :].T
        wv = Wv[c * HD:(c + 1) * HD, :].T
        wqkv_h = np.ascontiguousarray(
            np.concatenate([wq, wk, wv], axis=1).astype(np.float32))
        woT_h = np.ascontiguousarray(
            Wo[c * JQ:(c + 1) * JQ, :].T.astype(bf))
        in_maps.append({"xT": xTh, "wqkv": wqkv_h, "woT": woT_h, "tri": tri_h})
    return in_maps


def unshard(results):
    out = np.empty((S, D), dtype=np.float32)
    for c in range(NCORES):
        out[:, c * JQ:(c + 1) * JQ] = results[c]["out"].T
    return out.reshape(1, S, D)


def kernel(x, Wq, Wk, Wv, Wo):
    from concourse.bass_utils import run_bass_kernel_spmd
    nc = build_nc()
    in_maps = prep_in_maps(x, Wq, Wk, Wv, Wo)
    res = run_bass_kernel_spmd(nc, in_maps, core_ids=list(range(NCORES)))
    return unshard(res.results)
